# revision 20
# baseline (speedup 1.0000x reference)
"""Trainium2 Bass kernel for nn_LocalAggregation (ball-query KNN + grouped MLP + max-pool).

Math refactor: with BN in eval mode and ReLU/max commuting past the per-query
affine part, the whole conv+BN+ReLU+max collapses to
    out[c, m] = relu( max_{k in NN32(m)} Gt[k, c]  -  Ht[m, c] )
where Gt = (diag(s)@W) @ [fea; xyz/R]  per point, Ht per query,
s = gamma/sqrt(var+eps), and the ball-query mask replaces far neighbors with
the nearest (self) index.

Sharding: 8 cores = 4 batches x 2 query-halves. Each core handles 4096
queries x all 8192 keys of one batch sample.
"""

import numpy as np

import concourse.bacc as bacc
import concourse.bass as bass
import concourse.mybir as mybir
from concourse import tile
from concourse.bass import IndirectOffsetOnAxis
from concourse.bass_utils import run_bass_kernel_spmd

B, C, N = 4, 64, 8192
K = 32
RADIUS = 0.2
R2 = RADIUS * RADIUS
EPS = 1e-5
CIN = C + 3            # 67
NCORES = 8
QPC = N // 2           # queries per core
NT = QPC // 128        # query tiles per core (32)
NBLK = N // 128        # key blocks per tile (64)
NCAND = NBLK * 8       # candidates per query (512)
NEG = -3.0e38

f32 = mybir.dt.float32
u16 = mybir.dt.uint16
u32 = mybir.dt.uint32

_CACHE = {}


def _build(debug=False):
    nc = bacc.Bacc("TRN2", target_bir_lowering=False, debug=False,
                   num_devices=NCORES)

    xyz_in = nc.dram_tensor("xyz", [3, N], f32, kind="ExternalInput").ap()
    xyzq_in = nc.dram_tensor("xyzq", [3, QPC], f32, kind="ExternalInput").ap()
    fea_in = nc.dram_tensor("fea", [C, N], f32, kind="ExternalInput").ap()
    w_in = nc.dram_tensor("w", [C, CIN], f32, kind="ExternalInput").ap()
    bnt_in = nc.dram_tensor("bnt", [C, 4], f32, kind="ExternalInput").ap()
    blockbase_in = nc.dram_tensor("blockbase", [128, NCAND], u16,
                                  kind="ExternalInput").ap()
    ranks_in = nc.dram_tensor("ranks", [128, K], u16,
                              kind="ExternalInput").ap()
    constrows_in = nc.dram_tensor("constrows", [2, N], f32,
                                  kind="ExternalInput").ap()
    y_out = nc.dram_tensor("y", [QPC, C], f32, kind="ExternalOutput").ap()
    if debug:
        dbg = {
            "d_dist": nc.dram_tensor("d_dist", [128, N], f32, kind="ExternalOutput").ap(),
            "d_cand": nc.dram_tensor("d_cand", [128, NCAND], f32, kind="ExternalOutput").ap(),
            "d_gcand": nc.dram_tensor("d_gcand", [128, NCAND], u16, kind="ExternalOutput").ap(),
            "d_mxc": nc.dram_tensor("d_mxc", [128, K], f32, kind="ExternalOutput").ap(),
            "d_pos": nc.dram_tensor("d_pos", [128, K], u16, kind="ExternalOutput").ap(),
            "d_gidx": nc.dram_tensor("d_gidx", [128, K], u16, kind="ExternalOutput").ap(),
            "d_gath": nc.dram_tensor("d_gath", [128, K * C], f32, kind="ExternalOutput").ap(),
            "d_gt": nc.dram_tensor("d_gt", [N, C], f32, kind="ExternalOutput").ap(),
        }

    sq_dram = nc.dram_tensor("sq_scr", [1, N], f32).ap()
    nsq_dram = nc.dram_tensor("nsq_scr", [1, N], f32).ap()
    sqq_dram = nc.dram_tensor("sqq_scr", [1, QPC], f32).ap()
    cc_dram = nc.dram_tensor("cc_scr", [C, 1], f32).ap()
    wct_dram = nc.dram_tensor("wct_scr", [3, C], f32).ap()
    gt_dram = nc.dram_tensor("gt", [N, C], f32).ap()
    idx_dram = nc.dram_tensor("idxb", [NT * K * 128], u16).ap()

    with tile.TileContext(nc) as tc:
        # ---------------- persistent tiles ----------------
        with tc.tile_pool(name="persist", bufs=1) as pp:
            a5 = pp.tile([5, QPC], f32, tag="a5")        # query side lhsT rows
            a4 = pp.tile([4, QPC], f32, tag="a4")        # Ht lhsT rows
            b5 = pp.tile([5, N], f32, tag="b5")          # key side rhs rows
            rhs4 = pp.tile([4, C], f32, tag="rhs4")      # [Wc'/R ; cc] rhs
            blockbase = pp.tile([128, NCAND], u16, tag="bb")
            ranks = pp.tile([128, K], u16, tag="ranks")
            nc.sync.dma_start(out=blockbase[:], in_=blockbase_in[:])
            nc.sync.dma_start(out=ranks[:], in_=ranks_in[:])

            # ---------------- prep ----------------
            with tc.tile_pool(name="prep", bufs=1) as sp, \
                 tc.tile_pool(name="prep_ps", bufs=2, space="PSUM") as pps:
                f67 = sp.tile([CIN, N], f32)
                w = sp.tile([C, CIN], f32)
                bnt = sp.tile([C, 4], f32)
                nc.sync.dma_start(out=f67[:C, :], in_=fea_in[:])
                nc.sync.dma_start(out=f67[C:, :], in_=xyz_in[:])
                nc.sync.dma_start(out=w[:], in_=w_in[:])
                nc.sync.dma_start(out=bnt[:], in_=bnt_in[:])
                # load query coords early (a5 rows 0-2 double as pristine xyzq)
                nc.sync.dma_start(out=a5[0:3, :], in_=xyzq_in[:])
                nc.sync.dma_start(out=a4[0:3, :], in_=xyzq_in[:])

                # s = gamma / sqrt(var + eps); cc = s*mean - beta   (per channel)
                s_t = sp.tile([C, 1], f32)
                tmp = sp.tile([C, 1], f32)
                nc.vector.tensor_scalar_add(tmp[:], bnt[:, 3:4], EPS)
                nc.scalar.activation(tmp[:], tmp[:],
                                     mybir.ActivationFunctionType.Sqrt)
                nc.vector.reciprocal(tmp[:], tmp[:])
                nc.vector.tensor_mul(s_t[:], bnt[:, 0:1], tmp[:])
                cc_t = sp.tile([C, 1], f32)
                nc.vector.tensor_mul(cc_t[:], bnt[:, 2:3], s_t[:])
                nc.vector.tensor_sub(cc_t[:], cc_t[:], bnt[:, 1:2])
                nc.sync.dma_start(out=cc_dram[:], in_=cc_t[:])

                # W' = diag(s) @ W ; coor columns additionally * (1/R)
                wp = sp.tile([C, CIN], f32)
                nc.vector.tensor_scalar_mul(wp[:], w[:], s_t[:])
                nc.vector.tensor_scalar_mul(wp[:, C:], wp[:, C:], 1.0 / RADIUS)

                # diag(s') trick not needed: transpose W' via matmul with diag.
                # Instead compute W'T = lhsT(W').T @ I  using tensor.transpose
                # would need identity; cheaper: W'T[k, c] = sum_p W'[p, k] * D[p, c]
                diag = sp.tile([C, C], f32)
                nc.gpsimd.memset(diag[:], 0.0)
                one_col = sp.tile([C, 1], f32)
                nc.gpsimd.memset(one_col[:], 1.0)
                nc.gpsimd.affine_select(
                    diag[:], one_col[:].to_broadcast([C, C]),
                    pattern=[[-1, C]], base=0, channel_multiplier=1,
                    compare_op=mybir.AluOpType.is_equal, fill=0.0)
                wpt_ps = pps.tile([CIN, C], f32)
                nc.tensor.matmul(wpt_ps[:], wp[:], diag[:], start=True, stop=True)
                wpt = sp.tile([CIN, C], f32)
                nc.scalar.copy(wpt[:], wpt_ps[:])
                # stash coor rows of W'T for rhs4 assembly (partition shift via DRAM)
                nc.sync.dma_start(out=wct_dram[:], in_=wpt[C:, :])

                # sq = sum(xyz^2) along the 3 coords (PE ones-reduction), chunked
                ones3 = sp.tile([3, 1], f32)
                nc.gpsimd.memset(ones3[:], 1.0)
                for k in range(N // 512):
                    t3 = sp.tile([3, 512], f32, tag="t3")
                    nc.vector.tensor_mul(t3[:], f67[C:, k * 512:(k + 1) * 512],
                                         f67[C:, k * 512:(k + 1) * 512])
                    ps = pps.tile([1, 512], f32, tag="sqps")
                    nc.tensor.matmul(ps[:], ones3[:], t3[:], start=True, stop=True)
                    sqc = sp.tile([1, 512], f32, tag="sqc")
                    nc.scalar.copy(sqc[:], ps[:])
                    nc.sync.dma_start(out=sq_dram[:, k * 512:(k + 1) * 512],
                                      in_=sqc[:])
                    nsqc = sp.tile([1, 512], f32, tag="nsqc")
                    nc.vector.tensor_scalar_mul(nsqc[:], sqc[:], -1.0)
                    nc.sync.dma_start(out=nsq_dram[:, k * 512:(k + 1) * 512],
                                      in_=nsqc[:])
                for k in range(QPC // 512):
                    t3 = sp.tile([3, 512], f32, tag="t3")
                    nc.vector.tensor_mul(t3[:], a5[0:3, k * 512:(k + 1) * 512],
                                         a5[0:3, k * 512:(k + 1) * 512])
                    ps = pps.tile([1, 512], f32, tag="sqps")
                    nc.tensor.matmul(ps[:], ones3[:], t3[:], start=True, stop=True)
                    sqc = sp.tile([1, 512], f32, tag="sqc")
                    nc.scalar.copy(sqc[:], ps[:])
                    nc.sync.dma_start(out=sqq_dram[:, k * 512:(k + 1) * 512],
                                      in_=sqc[:])

                # B5 = [2x, 2y, 2z, -1, -sq] over keys
                nc.sync.dma_start(out=b5[0:3, :], in_=xyz_in[:])
                nc.vector.tensor_scalar_mul(b5[0:3, :], b5[0:3, :], 2.0)
                nc.sync.dma_start(out=b5[3:4, :], in_=constrows_in[1:2, :])
                nc.sync.dma_start(out=b5[4:5, :], in_=nsq_dram[:])

                # A5 rows: [x, y, z, sq, 1] over queries ; A4 row 3 = 1
                nc.sync.dma_start(out=a5[3:4, :], in_=sqq_dram[:])
                nc.sync.dma_start(out=a5[4:5, :], in_=constrows_in[0:1, :QPC])
                nc.sync.dma_start(out=a4[3:4, :], in_=constrows_in[0:1, :QPC])

                # rhs4 = [W'T coor rows ; ccT]
                nc.sync.dma_start(out=rhs4[0:3, :], in_=wct_dram[:])
                nc.sync.dma_start(out=rhs4[3:4, :],
                                  in_=cc_dram[:].rearrange("c one -> one c"))

                # Gt[n, c] = sum_p F67[p, n] * W'T[p, c]  -> DRAM [N, C]
                gstage = sp.tile([128, (N // 128) * C], f32)
                for blk in range(N // 128):
                    gps = pps.tile([128, C], f32, tag="gps")
                    nc.tensor.matmul(gps[:], f67[:, blk * 128:(blk + 1) * 128],
                                     wpt[:], start=True, stop=True)
                    nc.scalar.copy(gstage[:, blk * C:(blk + 1) * C], gps[:])
                nc.sync.dma_start(
                    out=gt_dram[:].rearrange("(blk p) c -> p blk c", p=128),
                    in_=gstage[:].rearrange("p (blk c) -> p blk c", c=C))

            # ---------------- phase A: selection over query tiles ----------------
            last_ls = None
            with tc.tile_pool(name="nd_ps", bufs=6, space="PSUM") as ndp, \
                 tc.tile_pool(name="dist", bufs=2) as dp, \
                 tc.tile_pool(name="small", bufs=2) as smp:
                for t in range(NT):
                    q0 = t * 128
                    dist = dp.tile([128, N], f32, tag="dist")
                    for k in range(N // 512):
                        ps = ndp.tile([128, 512], f32, tag="nd")
                        nc.tensor.matmul(ps[:], a5[:, q0:q0 + 128],
                                         b5[:, k * 512:(k + 1) * 512],
                                         start=True, stop=True)
                        nc.scalar.copy(dist[:, k * 512:(k + 1) * 512], ps[:])

                    cand = smp.tile([128, NCAND], f32, tag="cand")
                    lidx = smp.tile([128, NCAND], u16, tag="lidx")
                    for blk in range(NBLK):
                        dslice = dist[:, blk * 128:(blk + 1) * 128]
                        nc.vector.max(out=cand[:, blk * 8:blk * 8 + 8], in_=dslice)
                        nc.vector.max_index(out=lidx[:, blk * 8:blk * 8 + 8],
                                            in_max=cand[:, blk * 8:blk * 8 + 8],
                                            in_values=dslice)
                    gidx_cand = smp.tile([128, NCAND], u16, tag="gcand")
                    nc.vector.tensor_tensor(out=gidx_cand[:], in0=lidx[:],
                                            in1=blockbase[:],
                                            op=mybir.AluOpType.add)

                    # exact top-32 of the 512 candidates
                    work = smp.tile([128, NCAND], f32, tag="work")
                    mxc = smp.tile([128, K], f32, tag="mxc")
                    pos = smp.tile([128, K], u16, tag="pos")
                    src = cand
                    for it in range(4):
                        nc.vector.max(out=mxc[:, it * 8:it * 8 + 8], in_=src[:])
                        nc.vector.max_index(out=pos[:, it * 8:it * 8 + 8],
                                            in_max=mxc[:, it * 8:it * 8 + 8],
                                            in_values=src[:])
                        if it < 3:
                            nc.vector.match_replace(
                                out=work[:], in_to_replace=mxc[:, it * 8:it * 8 + 8],
                                in_values=src[:], imm_value=NEG)
                            src = work

                    # extract global idx at the 32 positions via two local_scatters
                    rank_at = smp.tile([128, NCAND], u16, tag="rank_at")
                    nc.gpsimd.local_scatter(
                        out_ap=rank_at[:], data_ap=ranks[:],
                        idxs_ap=pos[:].bitcast(mybir.dt.int16),
                        channels=128, num_elems=NCAND, num_idxs=K)
                    rankm1 = smp.tile([128, NCAND], mybir.dt.int16, tag="rankm1")
                    nc.vector.tensor_scalar(rankm1[:], rank_at[:], 1.0, None,
                                            op0=mybir.AluOpType.subtract)
                    gidx = smp.tile([128, K], u16, tag="gidx")
                    ls = nc.gpsimd.local_scatter(
                        out_ap=gidx[:], data_ap=gidx_cand[:], idxs_ap=rankm1[:],
                        channels=128, num_elems=K, num_idxs=NCAND)
                    last_ls = ls

                    # ball-query mask: slots with dist > R^2 (ndist < -R^2) -> idx0
                    mask = smp.tile([128, K], u32, tag="mask")
                    nc.vector.tensor_scalar(mask[:], mxc[:], -R2, None,
                                            op0=mybir.AluOpType.is_lt)
                    nc.vector.copy_predicated(gidx[:], mask[:],
                                              gidx[:, 0:1].to_broadcast([128, K]))
                    nc.sync.dma_start(
                        out=idx_dram[:].rearrange("(t s p) -> t p s", t=NT, s=K)[t],
                        in_=gidx[:])
                    if debug and t == 0:
                        nc.sync.dma_start(out=dbg["d_dist"][:], in_=dist[:])
                        nc.sync.dma_start(out=dbg["d_cand"][:], in_=cand[:])
                        nc.sync.dma_start(out=dbg["d_gcand"][:], in_=gidx_cand[:])
                        nc.sync.dma_start(out=dbg["d_mxc"][:], in_=mxc[:])
                        nc.sync.dma_start(out=dbg["d_pos"][:], in_=pos[:])
                        nc.sync.dma_start(out=dbg["d_gidx"][:], in_=gidx[:])

            # ---------------- phase B: gather + reduce ----------------
            with tc.tile_pool(name="h_ps", bufs=2, space="PSUM") as hps, \
                 tc.tile_pool(name="wrap", bufs=1) as wp2, \
                 tc.tile_pool(name="gath", bufs=2) as gp:
                # wrapped idx tile for all tiles, replicated into each
                # 16-partition group (one DMA per group)
                idxw_all = wp2.tile([128, NT * (K * 128 // 16)], u16, tag="idxw")
                for r in range(8):
                    nc.sync.dma_start(
                        out=idxw_all[r * 16:(r + 1) * 16, :].rearrange(
                            "w (t j) -> w t j", t=NT),
                        in_=idx_dram[:].rearrange("(t j w) -> w t j", t=NT, w=16))
                for t in range(NT):
                    q0 = t * 128
                    gath = gp.tile([128, K * C], f32, tag="gath")
                    dg = nc.gpsimd.dma_gather(
                        out_ap=gath[:].rearrange("p (s c) -> p s c", s=K),
                        in_ap=gt_dram[:],
                        idxs_ap=idxw_all[:, t * 256:(t + 1) * 256].bitcast(mybir.dt.int16),
                        num_idxs=K * 128, num_idxs_reg=K * 128, elem_size=C,
                        single_packet=False)
                    if last_ls is not None:
                        tile.add_dep_helper(
                            dg.ins, last_ls.ins, sync=False,
                            reason="keep mlp-library pool ops after local_scatter ops")

                    hp = hps.tile([128, C], f32, tag="hps")
                    nc.tensor.matmul(hp[:], a4[:, q0:q0 + 128], rhs4[:],
                                     start=True, stop=True)
                    ht = wp2.tile([128, C], f32, tag="ht")
                    nc.scalar.copy(ht[:], hp[:])

                    gmax = wp2.tile([128, C], f32, tag="gmax")
                    nc.vector.reduce_max(
                        out=gmax[:],
                        in_=gath[:].rearrange("p (s c) -> p c s", s=K),
                        axis=mybir.AxisListType.X)
                    o = wp2.tile([128, C], f32, tag="o")
                    nc.vector.tensor_sub(o[:], gmax[:], ht[:])
                    nc.vector.tensor_scalar_max(o[:], o[:], 0.0)
                    nc.sync.dma_start(out=y_out[q0:q0 + 128, :], in_=o[:])
                    if debug and t == 0:
                        nc.sync.dma_start(out=dbg["d_gath"][:], in_=gath[:])
                if debug:
                    nc.sync.dma_start(out=dbg["d_gt"][:], in_=gt_dram[:])

    nc.compile()
    return nc


def _get_nc():
    if "nc" not in _CACHE:
        _CACHE["nc"] = _build()
    return _CACHE["nc"]


def _make_in_maps(inputs):
    points_coor = np.ascontiguousarray(inputs["points_coor"], np.float32)
    points_fea = np.ascontiguousarray(inputs["points_fea"], np.float32)
    W = np.ascontiguousarray(inputs["W"], np.float32)
    bnt = np.ascontiguousarray(
        np.stack([inputs["gamma"], inputs["beta"], inputs["running_mean"],
                  inputs["running_var"]], axis=1), np.float32)
    blockbase = np.repeat((np.arange(NBLK, dtype=np.uint16) * 128), 8)
    blockbase = np.tile(blockbase[None, :], (128, 1)).copy()
    ranks = np.tile(np.arange(1, K + 1, dtype=np.uint16)[None, :], (128, 1)).copy()
    constrows = np.stack([np.ones(N, np.float32), -np.ones(N, np.float32)])
    in_maps = []
    for core in range(NCORES):
        b, h = core // 2, core % 2
        in_maps.append(dict(
            xyz=points_coor[b],
            xyzq=np.ascontiguousarray(points_coor[b][:, h * QPC:(h + 1) * QPC]),
            fea=points_fea[b],
            w=W,
            bnt=bnt,
            blockbase=blockbase,
            ranks=ranks,
            constrows=constrows,
        ))
    return in_maps


def kernel(points_coor, points_fea, W, gamma, beta, running_mean, running_var,
           **_unused):
    inputs = dict(points_coor=points_coor, points_fea=points_fea, W=W,
                  gamma=gamma, beta=beta, running_mean=running_mean,
                  running_var=running_var)
    nc = _get_nc()
    in_maps = _make_in_maps(inputs)
    res = run_bass_kernel_spmd(nc, in_maps, list(range(NCORES)))
    out = np.empty((B, C, N), np.float32)
    for core in range(NCORES):
        b, h = core // 2, core % 2
        out[b, :, h * QPC:(h + 1) * QPC] = res.results[core]["y"].T
    return out
